# revision 18
# baseline (speedup 1.0000x reference)
"""Child-Sum Tree-LSTM (reference nn_ChildSumTreeLSTM) on 8 Trainium2
NeuronCores via Bass/Tile, SPMD.

v2: features on SBUF partitions (transposed), nodes on the free dim.
Each core owns a contiguous slice of levels 5..8; leaves->level-5 is fully
core-local, levels 4..0 (341 nodes) finish on the host.

Layout trick: every level's h/c tile is stored *b-major* w.r.t. its parent
level (column = sibling_index * n_parents + parent_index). The permutation
is applied for free by the ACT sigmoid/tanh instructions (strided output
APs), so that
  - child-sum = two contiguous-half tensor adds
  - f*c group-sum = contiguous multiply + two contiguous-half adds
  - the f-gate fx term shares one contiguous rhs across the 4 sibling
    blocks (no broadcast-rhs matmuls, which stream ~2x slow)
  - all matmul rhs operands stay contiguous.
Matmuls are bf16 with fp32 PSUM accumulation; biases ride ACT.
"""
import sys
sys.path.insert(0, '/opt/trn_rl_repo')
import numpy as np
import ml_dtypes
import concourse.bacc as bacc
import concourse.mybir as mybir
from concourse.tile import TileContext
from concourse.alu_op_type import AluOpType

F32 = mybir.dt.float32
BF16 = mybir.dt.bfloat16
AFT = mybir.ActivationFunctionType
ALU = AluOpType
P = 128
NCORES = 8
BR = 4


def level_offs(D):
    return [(BR ** l - 1) // (BR - 1) for l in range(D + 1)]


def local_counts(D, cut):
    return {l: BR ** l // NCORES for l in range(cut, D + 1)}


def local_offs(D, cut):
    n = local_counts(D, cut)
    offs = {}
    acc = 0
    for l in range(cut, D + 1):
        offs[l] = acc
        acc += n[l]
    return offs, acc


def build_program(D, cut, c_dtype=BF16, debug_taps=False):
    nloc = local_counts(D, cut)
    loff, total_rows = local_offs(D, cut)
    CDT = c_dtype

    nc = bacc.Bacc("TRN2", target_bir_lowering=False, debug=False,
                   num_devices=NCORES)
    xT = nc.dram_tensor("xT", [2, P, total_rows], BF16, kind="ExternalInput")
    wx = nc.dram_tensor("wx", [2, P, 1024], BF16, kind="ExternalInput")
    wh = nc.dram_tensor("wh", [2, P, 1024], BF16, kind="ExternalInput")
    bias = nc.dram_tensor("bias", [P, 8], F32, kind="ExternalInput")
    ncut = nloc[cut]
    out_h = nc.dram_tensor("out_h", [2, P, ncut], BF16, kind="ExternalOutput")
    out_c = nc.dram_tensor("out_c", [2, P, ncut], CDT, kind="ExternalOutput")
    dbg = {}
    if debug_taps:
        for nm, shp in (("dbg_h8", [P, 2, 2048]), ("dbg_c8", [P, 2, 2048]),
                        ("dbg_hs7", [P, 2, 2048]), ("dbg_h7", [P, 2, 2048]),
                        ("dbg_c7", [P, 2, 2048]), ("dbg_ft7", [P, 2, 2048]),
                        ("dbg_fcs7", [P, 2, 512]),
                        ("dbg_h6", [P, 2, 512]), ("dbg_c6", [P, 2, 512]),
                        ("dbg_hs6", [P, 2, 512]), ("dbg_ft6", [P, 2, 2048]),
                        ("dbg_hs5", [P, 2, 128]),
                        ("dbg_it5", [P, 2, 128]), ("dbg_ot5", [P, 2, 128]),
                        ("dbg_ut5", [P, 2, 128]), ("dbg_fcs5", [P, 2, 128]),
                        ("dbg_fc5", [P, 2, 512]), ("dbg_ft5", [P, 2, 512])):
            dbg[nm] = nc.dram_tensor(nm, shp, BF16, kind="ExternalOutput")

    with TileContext(nc) as tc:
        with tc.tile_pool(name="const", bufs=1) as constp, \
             tc.tile_pool(name="xin", bufs=4) as xin, \
             tc.tile_pool(name="state", bufs=1) as statep, \
             tc.tile_pool(name="leafg", bufs=2) as leafg, \
             tc.tile_pool(name="work", bufs=2) as work, \
             tc.tile_pool(name="psum", bufs=1, space="PSUM") as psum:

            wxt = constp.tile([P, 2, 1024], BF16)
            wht = constp.tile([P, 2, 1024], BF16)
            bt = constp.tile([P, 8], F32)
            nc.sync.dma_start(wxt[:], wx[:].rearrange("a p n -> p a n"))
            nc.sync.dma_start(wht[:], wh[:].rearrange("a p n -> p a n"))
            nc.sync.dma_start(bt[:], bias[:])

            def load_x(l, c0, S, tag="xt"):
                t = xin.tile([P, 2, S], BF16, tag=tag, name=tag)
                src = xT[:, :, loff[l] + c0: loff[l] + c0 + S]
                nc.sync.dma_start(t[:], src.rearrange("a p n -> p a n"))
                return t

            # persistent level tiles; h/c are b-major w.r.t. parent level
            lt_h = {}
            lt_c = {}
            hs_t = {}
            for l in range(cut, D):
                lt_h[l] = statep.tile([P, 2, nloc[l]], BF16, name=f"h{l}")
                lt_c[l] = statep.tile([P, 2, nloc[l]], CDT, name=f"c{l}")
                hs_t[l] = statep.tile([P, 2, nloc[l]], BF16, name=f"hs{l}")

            def bm(ap):
                return ap.rearrange("p f (b i) -> p f b i", b=BR)

            def iou_psums(S):
                return [psum.tile([P, 2, S], F32, tag=t, name=t)
                        for t in ("pi", "po", "pu")]

            def iou_matmuls(xt, S, ps, hs=None):
                for g in range(3):
                    for ft in range(2):
                        mt = 2 * g + ft
                        dst = ps[g][:, ft, :]
                        nc.tensor.matmul(dst,
                                         wxt[:, 0, mt * P:(mt + 1) * P],
                                         xt[:, 0, :], start=True,
                                         stop=False)
                        last = hs is None
                        nc.tensor.matmul(dst, wxt[:, 1, mt * P:(mt + 1) * P],
                                         xt[:, 1, :], start=False, stop=last)
                        if hs is not None:
                            nc.tensor.matmul(dst,
                                             wht[:, 0, mt * P:(mt + 1) * P],
                                             hs[:, 0, :], start=False,
                                             stop=False)
                            nc.tensor.matmul(dst,
                                             wht[:, 1, mt * P:(mt + 1) * P],
                                             hs[:, 1, :], start=False,
                                             stop=True)

            def gates_bmajor(ps, S):
                """sigmoid/tanh psums -> b-major gate tiles [P, 2, S]."""
                it = work.tile([P, 2, S], BF16, tag="gi", name="gi")
                ot = work.tile([P, 2, S], BF16, tag="go", name="go")
                ut = work.tile([P, 2, S], BF16, tag="gu", name="gu")
                for g, (t, fn, b0) in enumerate(
                        ((it, AFT.Sigmoid, 0), (ot, AFT.Sigmoid, 2),
                         (ut, AFT.Tanh, 4))):
                    for ft in range(2):
                        dst = t[:, ft, :].rearrange("p (b i) -> p i b", b=BR)
                        src = ps[g][:, ft, :].rearrange("p (i b) -> p i b",
                                                        b=BR)
                        nc.scalar.activation(
                            dst, src, fn, bias=bt[:, b0 + ft:b0 + ft + 1])
                return it, ot, ut

            def finish_hc(it, ot, ut, S, h_dst, c_dst, fcs=None):
                """c = i*u (+ fcs); h = o*tanh(c). Gate tiles are b-major
                contiguous; h_dst/c_dst are 4D [P, 2, 4, S/4] views."""
                tmp = work.tile([P, 2, S], BF16, tag="tmp", name="tmp")
                with nc.allow_low_precision(reason="bf16 by design"):
                    if fcs is None:
                        nc.vector.tensor_tensor(c_dst, bm(it[:]), bm(ut[:]),
                                                ALU.mult)
                    else:
                        nc.vector.tensor_tensor(it[:], it[:], ut[:],
                                                ALU.mult)
                        nc.vector.tensor_tensor(c_dst, bm(it[:]), bm(fcs),
                                                ALU.add)
                    nc.scalar.activation(bm(tmp[:]), c_dst, AFT.Tanh)
                    nc.vector.tensor_tensor(h_dst, bm(ot[:]), bm(tmp[:]),
                                            ALU.mult)

            def leaf_chunk(k, s, h8, c8):
                """leaves [2048k+512s, +512) -> h8/c8 cols b*512+128s+i."""
                S = 512
                xt = load_x(D, 2048 * k + 512 * s, S, tag="xleaf")
                ps = iou_psums(S)
                iou_matmuls(xt, S, ps)
                it, ot, ut = gates_bmajor(ps, S)
                sub = S // BR
                hv = bm(h8[:])[:, :, :, sub * s:sub * (s + 1)]
                cv = bm(c8[:])[:, :, :, sub * s:sub * (s + 1)]
                finish_hc(it, ot, ut, S, hv, cv)

            def hsum(ch_ap, n4, dst):
                """dst[i] = sum_b ch[b*n4+i]; ch_ap [P,2,4*n4] b-major."""
                t = work.tile([P, 2, 2 * n4], BF16, tag="hst", name="hst")
                with nc.allow_low_precision(reason="bf16 by design"):
                    nc.gpsimd.tensor_add(t[:], ch_ap[:, :, 0:2 * n4],
                                         ch_ap[:, :, 2 * n4:4 * n4])
                    nc.gpsimd.tensor_add(dst, t[:, :, 0:n4],
                                         t[:, :, n4:2 * n4])

            def f_window(xt, S, ch_h, ft_tile, ftt, bp):
                """f-gate window: ftile ftt, sibling blocks {2bp, 2bp+1}.
                ch_h: [P, 2, 4S] b-major children h."""
                W = 2 * S
                psf = psum.tile([P, 1024], F32, tag="pf", name="pf")
                dst = psf[:, 0:W]
                wslc = slice(768 + ftt * P, 768 + (ftt + 1) * P)
                own_bank = S * 4 >= 2048   # each b-block fills a bank
                for kt in range(2):
                    rhs = ch_h[:, kt, bp * W:(bp + 1) * W]
                    for b in range(2):
                        # start=True clears the whole PSUM bank's has_written
                        # bits -> only the first matmul touching a bank may
                        # set it, or earlier blocks lose their partial sums
                        st = (kt == 0) and (b == 0 or own_bank)
                        nc.tensor.matmul(dst[:, b * S:(b + 1) * S],
                                         wht[:, kt, wslc],
                                         rhs[:, b * S:(b + 1) * S],
                                         start=st, stop=False)
                for kt in range(2):
                    for b in range(2):
                        nc.tensor.matmul(dst[:, b * S:(b + 1) * S],
                                         wxt[:, kt, wslc],
                                         xt[:, kt, :], start=False,
                                         stop=(kt == 1))
                nc.scalar.activation(
                    ft_tile[:, ftt, bp * W:(bp + 1) * W], dst, AFT.Sigmoid,
                    bias=bt[:, 6 + ftt:7 + ftt])

            def internal_tail(l, c0, S, ch_c, ft_tile, xt):
                """fc group-sum, iou, gates, h/c for parents [c0, c0+S)."""
                fc = work.tile([P, 2, BR * S], BF16, tag="fc", name="fc")
                t2 = work.tile([P, 2, 2 * S], BF16, tag="t2", name="t2")
                fcs = work.tile([P, 2, S], BF16, tag="fcs", name="fcs")
                with nc.allow_low_precision(reason="bf16 by design"):
                    nc.vector.tensor_tensor(fc[:], ft_tile[:], ch_c, ALU.mult)
                    nc.vector.tensor_tensor(t2[:], fc[:, :, 0:2 * S],
                                            fc[:, :, 2 * S:4 * S], ALU.add)
                    # last add permutes natural parent order -> b-major so
                    # fcs lines up with the b-major gate tiles
                    nc.vector.tensor_tensor(
                        fcs[:].rearrange("p f (b i) -> p f i b", b=BR),
                        t2[:, :, 0:S].rearrange("p f (i b) -> p f i b", b=BR),
                        t2[:, :, S:2 * S].rearrange("p f (i b) -> p f i b",
                                                    b=BR),
                        ALU.add)
                ps = iou_psums(S)
                iou_matmuls(xt, S, ps, hs=hs_t[l][:, :, c0:c0 + S])
                it, ot, ut = gates_bmajor(ps, S)
                nb = S // BR
                hv = bm(lt_h[l][:])[:, :, :, c0 // BR:c0 // BR + nb]
                cv = bm(lt_c[l][:])[:, :, :, c0 // BR:c0 // BR + nb]
                finish_hc(it, ot, ut, S, hv, cv, fcs=fcs[:])
                return it, ot, ut, fcs, fc

            # ================= emission =================
            # leaves fused with level D-1: groups of 2048 leaves
            lp = D - 1
            ngroups = nloc[lp] // 512
            pend = None
            for k in range(ngroups):
                h8 = leafg.tile([P, 2, 2048], BF16, tag="h8", name="h8")
                c8 = leafg.tile([P, 2, 2048], CDT, tag="c8", name="c8")
                for s in range(4):
                    leaf_chunk(k, s, h8, c8)
                    if pend is not None:
                        pk, ph8, pc8, pxt, pft = pend
                        f_window(pxt, 512, ph8[:], pft, s // 2, s % 2)
                hsum(h8[:], 512, hs_t[lp][:, :, 512 * k:512 * (k + 1)])
                if pend is not None:
                    pk, ph8, pc8, pxt, pft = pend
                    internal_tail(lp, 512 * pk, 512, pc8[:], pft, pxt)
                xt7 = load_x(lp, 512 * k, 512, tag="x7")
                ft7 = work.tile([P, 2, 2048], BF16, tag="ft", name="ft")
                pend = (k, h8, c8, xt7, ft7)
            pk, ph8, pc8, pxt, pft = pend
            for w in range(4):
                f_window(pxt, 512, ph8[:], pft, w // 2, w % 2)
            internal_tail(lp, 512 * pk, 512, pc8[:], pft, pxt)
            if debug_taps:
                nc.sync.dma_start(dbg["dbg_hs7"][:], hs_t[lp][:])
                nc.sync.dma_start(dbg["dbg_h7"][:], lt_h[lp][:])
                nc.sync.dma_start(dbg["dbg_c7"][:], lt_c[lp][:])
                nc.sync.dma_start(dbg["dbg_h8"][:], ph8[:])
                nc.sync.dma_start(dbg["dbg_c8"][:], pc8[:])
                nc.sync.dma_start(dbg["dbg_ft7"][:], pft[:])

            # levels D-2 .. cut (each one whole-level chunk)
            for l in range(D - 2, cut - 1, -1):
                S = nloc[l]
                hsum(lt_h[l + 1][:], S, hs_t[l][:])
                xt = load_x(l, 0, S)
                ftl = work.tile([P, 2, BR * S], BF16, tag="ft", name="ft")
                for w in range(4):
                    f_window(xt, S, lt_h[l + 1][:], ftl, w // 2, w % 2)
                tl = internal_tail(l, 0, S, lt_c[l + 1][:], ftl, xt)
                if debug_taps and l == 5:
                    it5, ot5, ut5, fcs5, fc5 = tl
                    for nm, t in (("dbg_it5", it5), ("dbg_ot5", ot5),
                                  ("dbg_ut5", ut5), ("dbg_fcs5", fcs5),
                                  ("dbg_fc5", fc5), ("dbg_ft5", ftl)):
                        nc.sync.dma_start(dbg[nm][:], t[:])
                if debug_taps and l == 6:
                    nc.sync.dma_start(dbg["dbg_h6"][:], lt_h[6][:])
                    nc.sync.dma_start(dbg["dbg_c6"][:], lt_c[6][:])
                    nc.sync.dma_start(dbg["dbg_hs6"][:], hs_t[6][:])
                    nc.sync.dma_start(dbg["dbg_ft6"][:], ftl[:])
                if debug_taps and l == 5:
                    nc.sync.dma_start(dbg["dbg_hs5"][:], hs_t[5][:])

            nc.sync.dma_start(out_h[:].rearrange("a p n -> p a n"),
                              lt_h[cut][:])
            nc.sync.dma_start(out_c[:].rearrange("a p n -> p a n"),
                              lt_c[cut][:])

    nc.compile()
    return nc


def shard_inputs(x, W_iou_x, b_iou_x, W_iou_h, b_iou_h, W_fx, b_fx, W_fh, b_fh,
                 D, cut):
    offs = level_offs(D)
    nloc = local_counts(D, cut)
    wx_cat = np.concatenate([W_iou_x, W_fx], axis=0)
    wh_cat = np.concatenate([W_iou_h, W_fh], axis=0)
    wx_d = np.ascontiguousarray(wx_cat.T).reshape(2, P, 1024).astype(
        ml_dtypes.bfloat16)
    wh_d = np.ascontiguousarray(wh_cat.T).reshape(2, P, 1024).astype(
        ml_dtypes.bfloat16)
    b_iou = (b_iou_x + b_iou_h).reshape(6, P).T
    b_f = (b_fx + b_fh).reshape(2, P).T
    bias = np.ascontiguousarray(
        np.concatenate([b_iou, b_f], axis=1)).astype(np.float32)
    in_maps = []
    for k in range(NCORES):
        rows = []
        for l in range(cut, D + 1):
            n = nloc[l]
            rows.append(x[offs[l] + k * n: offs[l] + (k + 1) * n])
        xl = np.concatenate(rows, axis=0)
        xTk = np.ascontiguousarray(xl.T).reshape(2, P, -1).astype(
            ml_dtypes.bfloat16)
        in_maps.append({"xT": xTk, "wx": wx_d, "wh": wh_d, "bias": bias})
    return in_maps


def finish_host(results, x, W_iou_x, b_iou_x, W_iou_h, b_iou_h,
                W_fx, b_fx, W_fh, b_fh, D, cut):
    ncut = BR ** cut
    npc = ncut // NCORES
    # device L5 tiles are b-major w.r.t. level 4: col = (j%4)*npar + j//4
    npar = npc // BR
    j = np.arange(npc)
    bcol = (j % BR) * npar + j // BR
    Hc = np.empty((ncut, 256), np.float32)
    Cc = np.empty((ncut, 256), np.float32)
    for k in range(NCORES):
        oh = results[k]["out_h"].astype(np.float32).reshape(256, npc)
        oc = results[k]["out_c"].astype(np.float32).reshape(256, npc)
        Hc[k * npc:(k + 1) * npc] = oh[:, bcol].T
        Cc[k * npc:(k + 1) * npc] = oc[:, bcol].T
    sig = lambda v: 1.0 / (1.0 + np.exp(-v))
    h_next, c_next = Hc, Cc
    for l in range(cut - 1, -1, -1):
        n, off = BR ** l, (BR ** l - 1) // 3
        xl = x[off:off + n]
        child_h = h_next.reshape(n, BR, 256)
        child_c = c_next.reshape(n, BR, 256)
        chs = child_h.sum(axis=1)
        iou = xl @ W_iou_x.T + b_iou_x + chs @ W_iou_h.T + b_iou_h
        i, o, u = np.split(iou, 3, axis=1)
        i, o, u = sig(i), sig(o), np.tanh(u)
        f = sig(child_h @ W_fh.T + b_fh + (xl @ W_fx.T + b_fx)[:, None, :])
        c = i * u + (f * child_c).sum(axis=1)
        h = o * np.tanh(c)
        h_next, c_next = h, c
    return c_next.astype(np.float32), h_next.astype(np.float32)


# ---------------- public API ----------------

_D = 8
_CUT = 5
_CACHE = {}


def _get_program():
    if "nc" not in _CACHE:
        _CACHE["nc"] = build_program(_D, _CUT)
    return _CACHE["nc"]


def kernel(x, W_iou_x, b_iou_x, W_iou_h, b_iou_h, W_fx, b_fx, W_fh, b_fh):
    from concourse import bass_utils
    x = np.asarray(x, dtype=np.float32)
    args = [np.asarray(a, dtype=np.float32) for a in
            (W_iou_x, b_iou_x, W_iou_h, b_iou_h, W_fx, b_fx, W_fh, b_fh)]
    nc = _get_program()
    in_maps = shard_inputs(x, *args, _D, _CUT)
    res = bass_utils.run_bass_kernel_spmd(nc, in_maps,
                                          core_ids=list(range(NCORES)))
    c, h = finish_host(res.results, x, *args, _D, _CUT)
    return c, h
